# revision 9
# baseline (speedup 1.0000x reference)
"""Trainium2 Bass kernel for nn_Net_39754217292108 (diffractive optical net).

Launch 1 (batch-sharded, 4 img/core): 6 diffraction steps as dense DFT
matmuls with pad/fftshift/crop folded into 400x200 / 200x400 DFT slices,
frequency-domain Hadamard with the transfer function, per-layer phase
modulation, |field|^2, and 8x8 avgpool via pooling matmuls.

Launch 2 (hidden-dim sharded, 64 h/core): the 625 sliding-window
BatchNorm+MLP heads, restructured: window masking is a banded 0/1 matrix
matmul against per-batch U^T*pooled_b; BatchNorm statistics + biases are
folded into host-computed rank-2 rows of the RHS. Partial logits summed
on host; the tiny vote/histogram assembly is numpy.
"""
import numpy as np
import concourse.bass as bass
import concourse.bacc as bacc
import concourse.mybir as mybir
from concourse.tile import TileContext
from concourse.bass_utils import run_bass_kernel_spmd

F32 = mybir.dt.float32
F32R = mybir.dt.float32r
ACTF = mybir.ActivationFunctionType

N_CORES = 8
BATCH = 32
N_IMG = BATCH // N_CORES          # 4 images per core
SIZE, PAD, PADDED = 200, 100, 400
P = 100                           # partition tile size for the 200/400 dims
M25, K8 = 25, 8
WIN = 5
HID, DIN, DOUT = 512, 625, 10
H_SH = HID // N_CORES             # 64 hidden units per core
EPS, THR, SHIFT, SCALE, EXT, T = 1e-5, 0.2, 0.0, 2.0, 1.0, 1.0
NCH = [125, 125, 125, 125, 125]   # K chunks for the 625-row classify matmul
L2N = [(0, 512), (512, 114)]      # N chunks (626-wide, even-padded)
DINP = 626                        # DIN padded to even for fp32r matmuls
M26 = 26                          # pooling cols padded even


# ---------------------------------------------------------------- host consts
def _static_consts():
    k = np.arange(PADDED, dtype=np.float64)
    c = np.arange(SIZE, dtype=np.float64)
    ang = np.outer(k, 300.0 + c) * (2.0 * np.pi / PADDED)
    A = np.exp(-1j * ang)                       # (400,200)
    B = (1.0 / PADDED) * np.exp(1j * ang).T     # (200,400) rows perm'd == A.T*  /400
    # B[i,k] = (1/400) exp(+2pi i (300+i)k/400); BT[k,i] = B[i,k] = A[k,i]*/400
    AT = A.T.copy()                             # (200,400)
    BT = B.T.copy()                             # (400,200)
    prt = np.zeros((SIZE, M26), np.float64)
    for mi in range(M25):
        prt[mi * K8:(mi + 1) * K8, mi] = 1.0 / K8
    # window mask (rc, w)
    r = np.arange(M25)
    ii, jj = np.meshgrid(r, r, indexing="ij")
    wi, wj = ii.ravel(), jj.ravel()
    rm = (r[None, :] >= wi[:, None]) & (r[None, :] < wi[:, None] + WIN)
    cm = (r[None, :] >= wj[:, None]) & (r[None, :] < wj[:, None] + WIN)
    Mmask = (rm[:, :, None] & cm[:, None, :]).reshape(DIN, DIN).T.astype(np.float64)
    f32 = lambda a: np.ascontiguousarray(a, np.float32)
    return dict(
        atr=f32(AT.real), ati=f32(AT.imag), atin=f32(-AT.imag),
        btr=f32(BT.real), bti=f32(BT.imag), btin=f32(-BT.imag),
        prt=f32(prt), mmask=Mmask,
    )


_SC = None


def _sc():
    global _SC
    if _SC is None:
        _SC = _static_consts()
    return _SC


# ---------------------------------------------------------------- launch 1 IR
def _build_diffract():
    nc = bacc.Bacc()
    x_in = nc.declare_dram_parameter("x", [N_IMG, SIZE, SIZE], F32R, isOutput=False)
    atr = nc.declare_dram_parameter("atr", [SIZE, PADDED], F32R, isOutput=False)
    ati = nc.declare_dram_parameter("ati", [SIZE, PADDED], F32R, isOutput=False)
    atin = nc.declare_dram_parameter("atin", [SIZE, PADDED], F32R, isOutput=False)
    btr = nc.declare_dram_parameter("btr", [PADDED, SIZE], F32R, isOutput=False)
    bti = nc.declare_dram_parameter("bti", [PADDED, SIZE], F32R, isOutput=False)
    btin = nc.declare_dram_parameter("btin", [PADDED, SIZE], F32R, isOutput=False)
    hr_d = nc.declare_dram_parameter("hr", [3, PADDED, PADDED], F32, isOutput=False)
    hi_d = nc.declare_dram_parameter("hi", [3, PADDED, PADDED], F32, isOutput=False)
    mr_d = nc.declare_dram_parameter("modr", [4, SIZE, SIZE], F32, isOutput=False)
    mi_d = nc.declare_dram_parameter("modi", [4, SIZE, SIZE], F32, isOutput=False)
    prt_d = nc.declare_dram_parameter("prt", [SIZE, M26], F32R, isOutput=False)
    pooled_d = nc.declare_dram_parameter("pooled", [N_IMG, M25, M25], F32, isOutput=True)

    r_ = lambda ap: ap.bitcast(F32R)

    with TileContext(nc) as tc:
        with (
            tc.tile_pool(name="cpool", bufs=1) as cp,
            tc.tile_pool(name="wpool", bufs=1) as wp,
            tc.tile_pool(name="pp", bufs=1, space="PSUM") as pp,
        ):
            def cload(src, nm, dt=F32):
                t = cp.tile(list(src.shape), dt, name=nm, tag=nm)
                nc.sync.dma_start(out=t, in_=src)
                return t

            at_t = {}
            for nm, d in (("atr", atr), ("ati", ati), ("atin", atin)):
                at_t[nm] = [cload(d[kc * P:(kc + 1) * P, :], f"{nm}{kc}", F32R) for kc in range(2)]
            bt_t = {}
            for nm, d in (("btr", btr), ("bti", bti), ("btin", btin)):
                bt_t[nm] = [cload(d[kc * P:(kc + 1) * P, :], f"{nm}{kc}", F32R) for kc in range(4)]
            hr_t = [[cload(hr_d[f, kc * P:(kc + 1) * P, :], f"hr{f}_{kc}") for kc in range(4)] for f in range(3)]
            hi_t = [[cload(hi_d[f, kc * P:(kc + 1) * P, :], f"hi{f}_{kc}") for kc in range(4)] for f in range(3)]
            mr_t = [[cload(mr_d[l, kc * P:(kc + 1) * P, :], f"mr{l}_{kc}") for kc in range(2)] for l in range(4)]
            mi_t = [[cload(mi_d[l, kc * P:(kc + 1) * P, :], f"mi{l}_{kc}") for kc in range(2)] for l in range(2 * 2)]
            prt_t = [cload(prt_d[kc * P:(kc + 1) * P, :], f"prt{kc}", F32R) for kc in range(2)]

            # load input images
            xs = {}
            for img in range(N_IMG):
                xr = []
                for kc in range(2):
                    t = wp.tile([P, SIZE], F32R, name=f"x0_{img}_{kc}", tag=f"xr{img}", bufs=3)
                    nc.sync.dma_start(out=t, in_=x_in[img, kc * P:(kc + 1) * P, :])
                    xr.append(t)
                xs[img] = (xr, None)

            uid = [0]

            def psum(nfree):
                uid[0] += 1
                return pp.tile([P, nfree], F32, name=f"ps{uid[0]}", tag="ps", bufs=8,
                               padded_shape=[128, 512])

            def sb(shape, tag, bufs, dt=F32):
                uid[0] += 1
                return wp.tile(list(shape), dt, name=f"t{uid[0]}", tag=tag, bufs=bufs)

            for step in range(6):
                hidx = 0 if step == 0 else (2 if step == 5 else 1)
                for img in range(N_IMG):
                    xr, xi = xs[img]
                    # ---- s1: out1[c,k] = sum_r X[r,c] AT[r,k]
                    o1r, o1i = [], []
                    for mt in range(2):
                        pr, pi = psum(PADDED), psum(PADDED)
                        pairs = []
                        for kc in range(2):
                            lr = xr[kc][:, mt * P:(mt + 1) * P]
                            pairs.append((lr, at_t["atr"][kc], at_t["ati"][kc]))
                            if xi is not None:
                                li = xi[kc][:, mt * P:(mt + 1) * P]
                                pairs.append((li, at_t["atin"][kc], at_t["atr"][kc]))
                        for n, (l, rr, ri) in enumerate(pairs):
                            nc.tensor.matmul(pr, r_(l), r_(rr), start=(n == 0), stop=(n == len(pairs) - 1))
                        for n, (l, rr, ri) in enumerate(pairs):
                            nc.tensor.matmul(pi, r_(l), r_(ri), start=(n == 0), stop=(n == len(pairs) - 1))
                        tr = sb((P, PADDED), "o1r", 4, F32R); ti = sb((P, PADDED), "o1i", 4, F32R)
                        nc.scalar.copy(tr, pr); nc.scalar.copy(ti, pi)
                        o1r.append(tr); o1i.append(ti)
                    # ---- s2: Y[k1,k2] = sum_c o1[c,k1] AT[c,k2];  Z = Y*H
                    zr, zi = [], []
                    for mt in range(4):
                        pr, pi = psum(PADDED), psum(PADDED)
                        sl = slice(mt * P, (mt + 1) * P)
                        pairs = []
                        for kc in range(2):
                            pairs.append((o1r[kc][:, sl], at_t["atr"][kc], at_t["ati"][kc]))
                            pairs.append((o1i[kc][:, sl], at_t["atin"][kc], at_t["atr"][kc]))
                        for n, (l, rr, ri) in enumerate(pairs):
                            nc.tensor.matmul(pr, r_(l), r_(rr), start=(n == 0), stop=(n == 3))
                        for n, (l, rr, ri) in enumerate(pairs):
                            nc.tensor.matmul(pi, r_(l), r_(ri), start=(n == 0), stop=(n == 3))
                        hr_, hi_ = hr_t[hidx][mt], hi_t[hidx][mt]
                        t1 = sb((P, PADDED), "tmp1", 3); t2 = sb((P, PADDED), "tmp2", 3)
                        t3 = sb((P, PADDED), "tmp3", 3); t4 = sb((P, PADDED), "tmp4", 3)
                        zr_ = sb((P, PADDED), "zr", 8, F32R); zi_ = sb((P, PADDED), "zi", 8, F32R)
                        nc.vector.tensor_mul(t1, pr, hr_)
                        nc.vector.tensor_mul(t2, pi, hi_)
                        nc.vector.tensor_sub(zr_, t1, t2)
                        nc.vector.tensor_mul(t3, pr, hi_)
                        nc.vector.tensor_mul(t4, pi, hr_)
                        nc.vector.tensor_add(zi_, t3, t4)
                        zr.append(zr_); zi.append(zi_)
                    # ---- s3: o3[k2,i] = sum_k1 Z[k1,k2] BT[k1,i]
                    o3r, o3i = [], []
                    for mt in range(4):
                        pr, pi = psum(SIZE), psum(SIZE)
                        sl = slice(mt * P, (mt + 1) * P)
                        pairs = []
                        for kc in range(4):
                            pairs.append((zr[kc][:, sl], bt_t["btr"][kc], bt_t["bti"][kc]))
                            pairs.append((zi[kc][:, sl], bt_t["btin"][kc], bt_t["btr"][kc]))
                        for n, (l, rr, ri) in enumerate(pairs):
                            nc.tensor.matmul(pr, r_(l), r_(rr), start=(n == 0), stop=(n == 7))
                        for n, (l, rr, ri) in enumerate(pairs):
                            nc.tensor.matmul(pi, r_(l), r_(ri), start=(n == 0), stop=(n == 7))
                        tr = sb((P, SIZE), "o3r", 8, F32R); ti = sb((P, SIZE), "o3i", 8, F32R)
                        nc.scalar.copy(tr, pr); nc.scalar.copy(ti, pi)
                        o3r.append(tr); o3i.append(ti)
                    # ---- s4: W[i,j] = sum_k2 o3[k2,i] BT[k2,j]; then mod / |.|^2
                    nxr, nxi, iout = [], [], []
                    for mt in range(2):
                        pr, pi = psum(SIZE), psum(SIZE)
                        sl = slice(mt * P, (mt + 1) * P)
                        pairs = []
                        for kc in range(4):
                            pairs.append((o3r[kc][:, sl], bt_t["btr"][kc], bt_t["bti"][kc]))
                            pairs.append((o3i[kc][:, sl], bt_t["btin"][kc], bt_t["btr"][kc]))
                        for n, (l, rr, ri) in enumerate(pairs):
                            nc.tensor.matmul(pr, r_(l), r_(rr), start=(n == 0), stop=(n == 7))
                        for n, (l, rr, ri) in enumerate(pairs):
                            nc.tensor.matmul(pi, r_(l), r_(ri), start=(n == 0), stop=(n == 7))
                        if step == 0:
                            a = sb((P, SIZE), f"xr{img}", 3, F32R); b = sb((P, SIZE), f"xi{img}", 3, F32R)
                            nc.scalar.copy(a, pr); nc.scalar.copy(b, pi)
                            nxr.append(a); nxi.append(b)
                        elif step < 5:
                            mr_, mi_ = mr_t[step - 1][mt], mi_t[step - 1][mt]
                            t1 = sb((P, SIZE), "tmp1", 3); t2 = sb((P, SIZE), "tmp2", 3)
                            t3 = sb((P, SIZE), "tmp3", 3); t4 = sb((P, SIZE), "tmp4", 3)
                            a = sb((P, SIZE), f"xr{img}", 3, F32R); b = sb((P, SIZE), f"xi{img}", 3, F32R)
                            nc.vector.tensor_mul(t1, pr, mr_)
                            nc.vector.tensor_mul(t2, pi, mi_)
                            nc.vector.tensor_sub(a, t1, t2)
                            nc.vector.tensor_mul(t3, pr, mi_)
                            nc.vector.tensor_mul(t4, pi, mr_)
                            nc.vector.tensor_add(b, t3, t4)
                            nxr.append(a); nxi.append(b)
                        else:
                            s1_ = sb((P, SIZE), "sq1", 3); s2_ = sb((P, SIZE), "sq2", 3)
                            io = sb((P, SIZE), "iout", 3, F32R)
                            nc.scalar.activation(s1_, pr, ACTF.Square)
                            nc.scalar.activation(s2_, pi, ACTF.Square)
                            nc.vector.tensor_add(io, s1_, s2_)
                            iout.append(io)
                    if step < 5:
                        xs[img] = (nxr, nxi)
                    else:
                        # ---- avgpool: p1[c,mi] = sum_r iout[r,c] prt[r,mi]
                        p1 = []
                        for mt in range(2):
                            ps_ = psum(M26)
                            for kc in range(2):
                                nc.tensor.matmul(ps_, r_(iout[kc][:, mt * P:(mt + 1) * P]),
                                                 r_(prt_t[kc]), start=(kc == 0), stop=(kc == 1))
                            t = sb((P, M26), "p1", 3, F32R)
                            nc.scalar.copy(t, ps_)
                            p1.append(t)
                        uid[0] += 1
                        ps2 = pp.tile([M25, M26], F32, name=f"pp{uid[0]}", tag="ps",
                                      bufs=8, padded_shape=[128, 512])
                        for kc in range(2):
                            nc.tensor.matmul(ps2, r_(p1[kc][:, 0:M25]), r_(prt_t[kc]),
                                             start=(kc == 0), stop=(kc == 1))
                        po = sb((M25, M25), "po", 3)
                        nc.scalar.copy(po, ps2[:, 0:M25])
                        nc.sync.dma_start(out=pooled_d[img], in_=po)
    return nc


# ---------------------------------------------------------------- launch 2 IR
def _build_classify():
    nc = bacc.Bacc()
    ut_d = nc.declare_dram_parameter("ut", [DIN, H_SH], F32R, isOutput=False)
    pt_d = nc.declare_dram_parameter("pooledT", [DIN, BATCH], F32, isOutput=False)
    sb_d = nc.declare_dram_parameter("s1b1", [2, H_SH], F32R, isOutput=False)
    rhs_d = nc.declare_dram_parameter("rhsf", [DIN + 2, DINP], F32R, isOutput=False)
    w2_d = nc.declare_dram_parameter("w2t", [H_SH, DOUT], F32R, isOutput=False)
    lg_d = nc.declare_dram_parameter("logits", [BATCH, DOUT, DIN], F32, isOutput=True)

    r_ = lambda ap: ap.bitcast(F32R)
    with TileContext(nc) as tc:
        with (
            tc.tile_pool(name="cpool", bufs=1) as cp,
            tc.tile_pool(name="wpool", bufs=1) as wp,
            tc.tile_pool(name="pp", bufs=1, space="PSUM") as pp,
        ):
            def cload(src, nm, dt=F32):
                t = cp.tile(list(src.shape), dt, name=nm, tag=nm)
                nc.sync.dma_start(out=t, in_=src)
                return t

            off = np.concatenate([[0], np.cumsum(NCH)])
            ut_t = [cload(ut_d[off[k]:off[k] + 125, :], f"ut{k}", F32R) for k in range(5)]
            pt_t = [cload(pt_d[off[k]:off[k] + 125, :], f"pt{k}") for k in range(5)]
            rhs_t = [cload(rhs_d[off[k]:off[k] + NCH[k], :], f"rhs{k}", F32R) for k in range(5)]
            rhs_sb = cload(rhs_d[DIN:DIN + 2, :], "rhssb", F32R)
            s1b1_t = cload(sb_d[:, :], "s1b1", F32R)
            w2t_t = cload(w2_d[:, :], "w2t", F32R)

            uid = [0]
            for b in range(BATCH):
                tts = []
                for kc in range(5):
                    uid[0] += 1
                    tt = wp.tile([125, H_SH], F32R, name=f"lh{uid[0]}", tag=f"lh{kc}", bufs=2)
                    nc.vector.tensor_scalar_mul(tt, ut_t[kc], pt_t[kc][:, b:b + 1])
                    tts.append(tt)
                uid[0] += 1
                hid = wp.tile([H_SH, DINP], F32R, name=f"hid{uid[0]}", tag="hid", bufs=2)
                for ni, (n0, nn) in enumerate(L2N):
                    uid[0] += 1
                    ps = pp.tile([H_SH, nn], F32, name=f"ps{uid[0]}", tag="ps", bufs=4,
                                 padded_shape=[128, 512])
                    for kc in range(5):
                        nc.tensor.matmul(ps, r_(tts[kc]), r_(rhs_t[kc][:, n0:n0 + nn]),
                                         start=(kc == 0), stop=False)
                    nc.tensor.matmul(ps, r_(s1b1_t), r_(rhs_sb[:, n0:n0 + nn]),
                                     start=False, stop=True)
                    nc.scalar.activation(hid[:, n0:n0 + nn], ps, ACTF.Relu)
                uid[0] += 1
                lg = wp.tile([DOUT, DINP], F32, name=f"lg{uid[0]}", tag="lg", bufs=2)
                for ni, (n0, nn) in enumerate(L2N):
                    uid[0] += 1
                    ps = pp.tile([DOUT, nn], F32, name=f"ps{uid[0]}", tag="ps2", bufs=4,
                                 padded_shape=[128, 512])
                    nc.tensor.matmul(ps, r_(w2t_t), r_(hid[:, n0:n0 + nn]), start=True, stop=True)
                    nc.scalar.copy(lg[:, n0:n0 + nn], ps)
                nc.sync.dma_start(out=lg_d[b], in_=lg[:, 0:DIN])
    return nc


_NC1, _NC2 = None, None
PROFILE = False            # test harness sets True to capture ntff traces
LAST = {}                  # exec_time_ns / trace dirs per launch


def _ncs():
    global _NC1, _NC2
    if _NC1 is None:
        _NC1 = _build_diffract()
        _NC1.finalize()
        _NC2 = _build_classify()
        _NC2.finalize()
    return _NC1, _NC2


# ---------------------------------------------------------------- host driver
def kernel(x, batch_size, h, h_pro, h_det, phase, W1, b1, W2, b2, gamma, beta):
    x = np.ascontiguousarray(np.asarray(x), np.float32)
    h = np.asarray(h); h_pro = np.asarray(h_pro); h_det = np.asarray(h_det)
    phase = np.asarray(phase, np.float64)
    W1 = np.asarray(W1, np.float64); b1 = np.asarray(b1, np.float64)
    W2 = np.asarray(W2, np.float64); b2 = np.asarray(b2, np.float64)
    gamma = float(np.asarray(gamma).ravel()[0]); beta = float(np.asarray(beta).ravel()[0])
    sc = _sc()
    nc1, nc2 = _ncs()
    f32 = lambda a: np.ascontiguousarray(a, np.float32)

    theta = SCALE * np.pi * (np.sin(EXT * phase) + 1.0)
    hs = np.stack([h_pro, h, h_det])
    common = dict(
        atr=sc["atr"], ati=sc["ati"], atin=sc["atin"],
        btr=sc["btr"], bti=sc["bti"], btin=sc["btin"],
        hr=f32(hs.real), hi=f32(hs.imag),
        modr=f32(np.cos(theta)), modi=f32(np.sin(theta)),
        prt=sc["prt"],
    )
    in_maps = [dict(common, x=x[c * N_IMG:(c + 1) * N_IMG]) for c in range(N_CORES)]
    kw1 = dict(trace=True, tmpdir="/tmp/prof_l1") if PROFILE else {}
    br1 = run_bass_kernel_spmd(nc1, in_maps, list(range(N_CORES)), **kw1)
    LAST["l1"] = br1
    res1 = br1.results
    pooled = np.concatenate([r["pooled"] for r in res1], axis=0)   # (32,25,25)

    # ---- host: fold BN stats into classify inputs
    Pf = pooled.reshape(BATCH, DIN).astype(np.float64)
    pb = Pf.sum(0).reshape(M25, M25)
    pb2 = (Pf ** 2).sum(0).reshape(M25, M25)

    def boxsum(a):
        o = np.zeros((M25, M25))
        for i in range(M25):
            for j in range(M25):
                o[i, j] = a[i:i + WIN, j:j + WIN].sum()
        return o
    NT = BATCH * DIN
    mu = boxsum(pb).ravel() / NT
    var = boxsum(pb2).ravel() / NT - mu ** 2
    sd = np.sqrt(var + EPS)
    C1 = gamma / sd
    cw = beta - gamma * mu / sd
    s1 = W1.sum(axis=1)
    rhsf = np.concatenate([sc["mmask"] * C1[None, :], cw[None, :],
                           np.ones((1, DIN))], axis=0)
    rhsf = f32(np.pad(rhsf, ((0, 0), (0, DINP - DIN))))
    ptT = f32(Pf.T)                                                # (625,32)
    in_maps2 = []
    for c in range(N_CORES):
        rows = slice(c * H_SH, (c + 1) * H_SH)
        in_maps2.append(dict(
            ut=f32(W1[rows].T), pooledT=ptT,
            s1b1=f32(np.stack([s1[rows], b1[rows]])),
            rhsf=rhsf, w2t=f32(W2[:, rows].T),
        ))
    kw2 = dict(trace=True, tmpdir="/tmp/prof_l2") if PROFILE else {}
    br2 = run_bass_kernel_spmd(nc2, in_maps2, list(range(N_CORES)), **kw2)
    LAST["l2"] = br2
    res2 = br2.results
    logits = np.sum([r["logits"] for r in res2], axis=0).astype(np.float64)
    logits += b2[None, :, None]
    logits /= T

    # ---- vote / histogram assembly
    e = np.exp(logits - logits.max(axis=1, keepdims=True))
    p = e / e.sum(axis=1, keepdims=True)                           # (32,10,625)
    a = p.argmax(axis=1)                                           # (32,625)
    delta = p.max(axis=1) - p.min(axis=1)
    counts = (a[:, :, None] == np.arange(DOUT + 1)[None, None, :]).sum(0)  # (625,11)
    final0 = counts.argmax(axis=1)                                 # (625,)
    dm = delta * (a == final0[None, :])
    nz = (dm != 0).sum(0)
    with np.errstate(invalid="ignore"):
        delta1 = np.where(nz > 0, dm.sum(0) / np.maximum(nz, 1), np.nan)
    final = np.where(delta1 < THR, 0, final0 + 1).astype(np.int32)
    result = np.where(final == 0, 0.0, final.astype(np.float32) + SHIFT)
    ch_map = (final != 0).astype(np.float32)
    histo = np.bincount(final, minlength=DOUT + 1).astype(np.float32)
    return (result.reshape(M25, M25).astype(np.float32),
            ch_map.reshape(M25, M25),
            histo)
